# revision 2
# baseline (speedup 1.0000x reference)
"""AdaLoRA dense MLP kernel for 8 TRN2 NeuronCores.

out = x @ (W + (alpha/r) * (P * sigma) @ Q^T)^T

The LoRA delta is rank-12: merging it into W on the host costs ~400
MFLOP of numpy and removes the z-phase + augmented-K matmuls from the
device entirely. The device then runs one dense bf16 GEMM per core,
data-parallel over the 16384 token rows (2048/core, no collectives).

Per-core blocking (M=2048, K=4096, N=4096), x stationary / W moving:
  xT resident in SBUF: 32 K-tiles [128, 2048] bf16 (128 KB/partition)
  for n in 8:                 # 512-wide output chunks; ws strips stream
    for g in 2:               # 8 m-tiles -> all 8 PSUM banks
      for k in 32: 8 matmuls (N=512) accumulating
      evict: vector/scalar copy PSUM->SBUF, DMA out on sync queue
Eviction of bank b overlaps the tail of the k-loop (per-bank deps), so
the PE stream is 4096 back-to-back N=512 matmuls at the 216 ns floor
(~885 us); chunk 0 paces the xT+ws input DMA at ~97% PE busy.

DMA: xT strips (512 KB, contiguous) alternate sync/scalar queues; wT
strips (128 KB, contiguous via host marshal) on gpsimd; output on sync.
"""

import numpy as np

B, S, IN_F, OUT_F, RANK = 4, 4096, 4096, 4096, 12
SCALING = 16.0 / 12.0
N_CORES = 8
M_TOTAL = B * S               # 16384
M_CORE = M_TOTAL // N_CORES   # 2048

P = 128                       # partitions
K_TILES = IN_F // P           # 32
N_CHUNK = 512
N_CHUNKS = OUT_F // N_CHUNK   # 8
M_TILES = M_CORE // P         # 16
MG = 8                        # m-tiles per PSUM group (all 8 banks)
M_GROUPS = M_TILES // MG      # 2

_CACHE = {}


def _build():
    import concourse.bass as bass
    import concourse.tile as tile
    from concourse import bacc, mybir

    BF = mybir.dt.bfloat16
    F32 = mybir.dt.float32

    nc = bacc.Bacc("TRN2", target_bir_lowering=False, debug=False,
                   num_devices=N_CORES)

    xT = nc.declare_dram_parameter("xT", [IN_F, M_CORE], BF, isOutput=False)
    # host marshals wT so each [128, 512] strip is contiguous:
    # row ((n*32 + k)*128 + p), col c  ->  Wm[n*512+c, k*128+p]
    wT = nc.declare_dram_parameter(
        "wT", [N_CHUNKS * K_TILES * P, N_CHUNK], BF, isOutput=False)
    # out row ((mt*8 + n)*128 + p), col c -> out[mt*128+p, n*512+c]
    out = nc.declare_dram_parameter(
        "out", [M_TILES * N_CHUNKS * P, N_CHUNK], F32, isOutput=True)

    xT_ap, wT_ap, out_ap = xT.ap(), wT.ap(), out.ap()

    with tile.TileContext(nc) as tc:
        with tc.tile_pool(name="xs", bufs=K_TILES) as xs_pool, \
             tc.tile_pool(name="ws", bufs=K_TILES + 4) as ws_pool, \
             tc.tile_pool(name="stage", bufs=8) as stage_pool, \
             tc.tile_pool(name="ps", bufs=8, space="PSUM") as ps_pool:

            # resident x: 32 strips [128, 2048] bf16, each one contiguous
            # 512 KB DMA; alternate queues to double early issue rate
            xs_tiles = []
            for k in range(K_TILES):
                xst = xs_pool.tile([P, M_CORE], BF, tag="xs", name=f"xs_{k}")
                eng = nc.sync if k % 2 == 0 else nc.scalar
                eng.dma_start(out=xst[:], in_=xT_ap[bass.ts(k, P), :])
                xs_tiles.append(xst)

            for n in range(N_CHUNKS):
                ws_tiles = []
                for k in range(K_TILES):
                    wst = ws_pool.tile([P, N_CHUNK], BF, tag="ws",
                                       name=f"ws_{n}_{k}")
                    nc.gpsimd.dma_start(
                        out=wst[:],
                        in_=wT_ap[bass.ts(n * K_TILES + k, P), :])
                    ws_tiles.append(wst)

                for g in range(M_GROUPS):
                    ps = [ps_pool.tile([P, N_CHUNK], F32, tag="ps",
                                       name=f"ps_{n}_{g}_{mi}")
                          for mi in range(MG)]
                    for k in range(K_TILES):
                        for mi in range(MG):
                            nc.tensor.matmul(
                                ps[mi][:],
                                lhsT=xs_tiles[k][:, bass.ds((g * MG + mi) * P, P)],
                                rhs=ws_tiles[k][:],
                                start=(k == 0),
                                stop=(k == K_TILES - 1),
                            )
                    for mi in range(MG):
                        m_tile = g * MG + mi
                        st = stage_pool.tile([P, N_CHUNK], F32, tag="st",
                                             name=f"st_{n}_{g}_{mi}")
                        # split evictions across two engines so bank 7 is
                        # clear before the next group's k=0 matmul needs it
                        if mi % 2 == 0:
                            nc.vector.tensor_copy(out=st[:], in_=ps[mi][:])
                        else:
                            nc.scalar.copy(out=st[:], in_=ps[mi][:])
                        nc.sync.dma_start(
                            out=out_ap[bass.ts(m_tile * N_CHUNKS + n, P), :],
                            in_=st[:])

    nc.compile()
    return nc


def _get_nc():
    if "nc" not in _CACHE:
        _CACHE["nc"] = _build()
    return _CACHE["nc"]


def _marshal(x, weight, lora_P, lora_sigma, lora_Q):
    import ml_dtypes

    bf16 = ml_dtypes.bfloat16
    W = np.asarray(weight, dtype=np.float32)
    Ps = np.asarray(lora_P, dtype=np.float32) * np.asarray(
        lora_sigma, dtype=np.float32)[None, :]
    Wm = W + SCALING * (Ps @ np.asarray(lora_Q, dtype=np.float32).T)
    # [in, out] -> [n, k, p, c] -> 2D so each [128, 512] strip is contiguous
    wT_np = np.ascontiguousarray(
        Wm.T.astype(bf16)
        .reshape(K_TILES, P, N_CHUNKS, N_CHUNK)
        .transpose(2, 0, 1, 3)
        .reshape(N_CHUNKS * K_TILES * P, N_CHUNK))
    X = np.asarray(x, dtype=np.float32).reshape(M_TOTAL, IN_F)
    in_maps = []
    for c in range(N_CORES):
        xT_np = np.ascontiguousarray(
            X[c * M_CORE:(c + 1) * M_CORE].T.astype(bf16))
        in_maps.append({"xT": xT_np, "wT": wT_np})
    return in_maps


def _gather(res):
    outs = []
    for c in range(N_CORES):
        o = np.asarray(res.results[c]["out"])
        outs.append(
            o.reshape(M_TILES, N_CHUNKS, P, N_CHUNK)
            .transpose(0, 2, 1, 3)
            .reshape(M_CORE, OUT_F))
    return np.concatenate(outs, axis=0).reshape(B, S, OUT_F)


def kernel(x, weight, lora_P, lora_sigma, lora_Q):
    from concourse.bass_utils import run_bass_kernel_spmd

    nc = _get_nc()
    in_maps = _marshal(x, weight, lora_P, lora_sigma, lora_Q)
    res = run_bass_kernel_spmd(nc, in_maps, core_ids=list(range(N_CORES)))
    return _gather(res)


# revision 3
# speedup vs baseline: 1.1571x; 1.1571x over previous
"""AdaLoRA dense MLP kernel for 8 TRN2 NeuronCores.

out = x @ (W + (alpha/r) * (P * sigma) @ Q^T)^T

The LoRA delta is rank-12: merging it into W on the host costs ~400
MFLOP of numpy and removes the z-phase + augmented-K matmuls from the
device entirely. The device then runs one dense bf16 GEMM per core,
data-parallel over the 16384 token rows (2048/core, no collectives).

Per-core blocking (M=2048, K=4096, N=4096), x stationary / W moving:
  xT resident in SBUF as 32x8 strips [128, 256] bf16 (128 KB/partition)
  for n in 4:               # 1024-wide output chunks; ws strips stream
    for g in 8:             # 2 m-tiles x 2 n-halves -> 4 PSUM banks,
                            # ping-pong so evictions overlap fully
      for k in 32: 4 matmuls (N=512), each stationary x-tile reused
        for both n-halves -> LDWEIGHTS amortized 2x (an un-amortized
        LDWEIGHTS adds +43ns to every 216ns matmul: measured 259ns)
      evict: vector/scalar copy PSUM->SBUF bf16, DMA out on sync queue
The PE stream is 4096 back-to-back N=512 matmuls at the 216 ns floor
(~885 us); chunk 0 paces the xT+ws input DMA at ~95% PE busy.

DMA queues: xs eighth-strips (64 KB, contiguous) alternate sync/scalar
in need order; wT strips (256 KB, contiguous via host marshal) on
gpsimd; output (bf16, upcast on host) on sync.
"""

import numpy as np

B, S, IN_F, OUT_F, RANK = 4, 4096, 4096, 4096, 12
SCALING = 16.0 / 12.0
N_CORES = 8
M_TOTAL = B * S               # 16384
M_CORE = M_TOTAL // N_CORES   # 2048

P = 128                       # partitions
K_TILES = IN_F // P           # 32
N_CHUNK = 1024                # ws chunk width
N_CHUNKS = OUT_F // N_CHUNK   # 4
NH = 512                      # psum half width
M_TILES = M_CORE // P         # 16
MG = 2                        # m-tiles per group (2mi x 2h = 4 banks)
M_GROUPS = M_TILES // MG      # 8
ME = 256                      # xs strip width (one m-group)

_CACHE = {}


def _build():
    import concourse.bass as bass
    import concourse.tile as tile
    from concourse import bacc, mybir

    BF = mybir.dt.bfloat16
    F32 = mybir.dt.float32

    nc = bacc.Bacc("TRN2", target_bir_lowering=False, debug=False,
                   num_devices=N_CORES)

    xT = nc.declare_dram_parameter("xT", [IN_F, M_CORE], BF, isOutput=False)
    # host marshals wT so each [128, 1024] strip is contiguous:
    # row ((n*32 + k)*128 + p), col c  ->  Wm[n*1024+c, k*128+p]
    wT = nc.declare_dram_parameter(
        "wT", [N_CHUNKS * K_TILES * P, N_CHUNK], BF, isOutput=False)
    # out row ((mt*8 + nh)*128 + p), col c -> out[mt*128+p, nh*512+c], bf16
    out = nc.declare_dram_parameter(
        "out", [M_TILES * (OUT_F // NH) * P, NH], BF, isOutput=True)

    xT_ap, wT_ap, out_ap = xT.ap(), wT.ap(), out.ap()

    with tile.TileContext(nc) as tc:
        with tc.tile_pool(name="xs", bufs=K_TILES * M_GROUPS) as xs_pool, \
             tc.tile_pool(name="ws", bufs=K_TILES + 2) as ws_pool, \
             tc.tile_pool(name="stage", bufs=8) as stage_pool, \
             tc.tile_pool(name="ps", bufs=8, space="PSUM") as ps_pool:

            # resident x: 32x8 strips [128, 256] bf16, each a contiguous
            # 64 KB DMA; emitted in need order (m-group-major, k-minor)
            # split across the sync/scalar queues
            xs_tiles = [[None] * K_TILES for _ in range(M_GROUPS)]
            for e in range(M_GROUPS):
                for k in range(K_TILES):
                    xst = xs_pool.tile([P, ME], BF, tag="xs",
                                       name=f"xs_{e}_{k}")
                    eng = nc.sync if e % 2 == 0 else nc.scalar
                    eng.dma_start(
                        out=xst[:],
                        in_=xT_ap[bass.ts(k, P), bass.ts(e, ME)])
                    xs_tiles[e][k] = xst

            for n in range(N_CHUNKS):
                ws_tiles = []
                for k in range(K_TILES):
                    wst = ws_pool.tile([P, N_CHUNK], BF, tag="ws",
                                       name=f"ws_{n}_{k}")
                    nc.gpsimd.dma_start(
                        out=wst[:],
                        in_=wT_ap[bass.ts(n * K_TILES + k, P), :])
                    ws_tiles.append(wst)

                for g in range(M_GROUPS):
                    ps = [[ps_pool.tile([P, NH], F32, tag="ps",
                                        name=f"ps_{n}_{g}_{mi}_{h}")
                           for h in range(2)] for mi in range(MG)]
                    for k in range(K_TILES):
                        for mi in range(MG):
                            for h in range(2):
                                nc.tensor.matmul(
                                    ps[mi][h][:],
                                    lhsT=xs_tiles[g][k][:, bass.ts(mi, P)],
                                    rhs=ws_tiles[k][:, bass.ts(h, NH)],
                                    start=(k == 0),
                                    stop=(k == K_TILES - 1),
                                )
                    for mi in range(MG):
                        for h in range(2):
                            m_tile = g * MG + mi
                            nh = n * 2 + h
                            st = stage_pool.tile([P, NH], BF, tag="st",
                                                 name=f"st_{n}_{g}_{mi}_{h}")
                            if (mi + h) % 2 == 0:
                                nc.vector.tensor_copy(out=st[:],
                                                      in_=ps[mi][h][:])
                            else:
                                nc.scalar.copy(out=st[:], in_=ps[mi][h][:])
                            nc.sync.dma_start(
                                out=out_ap[
                                    bass.ts(m_tile * (OUT_F // NH) + nh, P),
                                    :],
                                in_=st[:])

    nc.compile()
    return nc


def _get_nc():
    if "nc" not in _CACHE:
        _CACHE["nc"] = _build()
    return _CACHE["nc"]


def _marshal(x, weight, lora_P, lora_sigma, lora_Q):
    import ml_dtypes

    bf16 = ml_dtypes.bfloat16
    W = np.asarray(weight, dtype=np.float32)
    Ps = np.asarray(lora_P, dtype=np.float32) * np.asarray(
        lora_sigma, dtype=np.float32)[None, :]
    Wm = W + SCALING * (Ps @ np.asarray(lora_Q, dtype=np.float32).T)
    # [in, out] -> [n, k, p, c] -> 2D so each [128, 1024] strip is contiguous
    wT_np = np.ascontiguousarray(
        Wm.T.astype(bf16)
        .reshape(K_TILES, P, N_CHUNKS, N_CHUNK)
        .transpose(2, 0, 1, 3)
        .reshape(N_CHUNKS * K_TILES * P, N_CHUNK))
    X = np.asarray(x, dtype=np.float32).reshape(M_TOTAL, IN_F)
    in_maps = []
    for c in range(N_CORES):
        xT_np = np.ascontiguousarray(
            X[c * M_CORE:(c + 1) * M_CORE].T.astype(bf16))
        in_maps.append({"xT": xT_np, "wT": wT_np})
    return in_maps


def _gather(res):
    outs = []
    for c in range(N_CORES):
        o = np.asarray(res.results[c]["out"]).astype(np.float32)
        outs.append(
            o.reshape(M_TILES, OUT_F // NH, P, NH)
            .transpose(0, 2, 1, 3)
            .reshape(M_CORE, OUT_F))
    return np.concatenate(outs, axis=0).reshape(B, S, OUT_F)


def kernel(x, weight, lora_P, lora_sigma, lora_Q):
    from concourse.bass_utils import run_bass_kernel_spmd

    nc = _get_nc()
    in_maps = _marshal(x, weight, lora_P, lora_sigma, lora_Q)
    res = run_bass_kernel_spmd(nc, in_maps, core_ids=list(range(N_CORES)))
    return _gather(res)


# revision 7
# speedup vs baseline: 1.1870x; 1.0258x over previous
"""AdaLoRA dense MLP kernel for 8 TRN2 NeuronCores.

out = x @ (W + (alpha/r) * (P * sigma) @ Q^T)^T

The LoRA delta is rank-12: merging it into W on the host costs ~400
MFLOP of numpy and removes the z-phase + augmented-K matmuls from the
device entirely. The device then runs one dense bf16 GEMM per core,
data-parallel over the 16384 token rows (2048/core, no collectives).

Per-core blocking (M=2048, K=4096, N=4096), x stationary / W moving:
  xT resident in SBUF as 32x8 strips [128, 256] bf16 (128 KB/partition)
  for n in 4:               # 1024-wide output chunks; ws strips stream
    for g in 8:             # 2 m-tiles x 2 n-halves -> 4 PSUM banks,
                            # ping-pong so evictions overlap fully
      for k in 32: 4 matmuls (N=512), each stationary x-tile reused
        for both n-halves -> LDWEIGHTS amortized 2x (an un-amortized
        LDWEIGHTS adds +43ns to every 216ns matmul: measured 259ns)
      evict: vector/scalar copy PSUM->SBUF bf16, DMA out on sync queue
The PE stream is 4096 back-to-back N=512 matmuls at the 216 ns floor
(~885 us); chunk 0 paces the xT+ws input DMA at ~95% PE busy.

DMA queues: xs eighth-strips (64 KB, contiguous) alternate sync/scalar
in need order; wT strips (256 KB, contiguous via host marshal) on
gpsimd; output (bf16, upcast on host) on sync.
"""

import numpy as np

B, S, IN_F, OUT_F, RANK = 4, 4096, 4096, 4096, 12
SCALING = 16.0 / 12.0
N_CORES = 8
M_TOTAL = B * S               # 16384
M_CORE = M_TOTAL // N_CORES   # 2048

P = 128                       # partitions
K_TILES = IN_F // P           # 32
N_CHUNK = 1024                # ws chunk width
N_CHUNKS = OUT_F // N_CHUNK   # 4
NH = 512                      # psum half width
M_TILES = M_CORE // P         # 16
MG = 2                        # m-tiles per group (2mi x 2h = 4 banks)
M_GROUPS = M_TILES // MG      # 8
ME = 256                      # xs strip width (one m-group)

_CACHE = {}


def _build():
    import concourse.bass as bass
    import concourse.tile as tile
    from concourse import bacc, mybir

    BF = mybir.dt.bfloat16
    F32 = mybir.dt.float32

    nc = bacc.Bacc("TRN2", target_bir_lowering=False, debug=False,
                   num_devices=N_CORES)

    xT = nc.declare_dram_parameter("xT", [IN_F, M_CORE], BF, isOutput=False)
    # host marshals wT so each [128, 1024] strip is contiguous:
    # row ((n*32 + k)*128 + p), col c  ->  Wm[n*1024+c, k*128+p]
    wT = nc.declare_dram_parameter(
        "wT", [N_CHUNKS * K_TILES * P, N_CHUNK], BF, isOutput=False)
    # out row ((mt*8 + nh)*128 + p), col c -> out[mt*128+p, nh*512+c], bf16
    out = nc.declare_dram_parameter(
        "out", [M_TILES * (OUT_F // NH) * P, NH], BF, isOutput=True)

    xT_ap, wT_ap, out_ap = xT.ap(), wT.ap(), out.ap()

    with tile.TileContext(nc) as tc:
        with tc.tile_pool(name="xs", bufs=K_TILES * M_GROUPS) as xs_pool, \
             tc.tile_pool(name="ws", bufs=K_TILES + 2) as ws_pool, \
             tc.tile_pool(name="stage", bufs=8) as stage_pool, \
             tc.tile_pool(name="ps", bufs=8, space="PSUM") as ps_pool:

            # resident x: 32x8 strips [128, 256] bf16, each a contiguous
            # 64 KB DMA; emitted in need order (m-group-major, k-minor),
            # all on the scalar queue (nothing else rides it, so the
            # eviction path never queues behind these)
            xs_tiles = [[None] * K_TILES for _ in range(M_GROUPS)]
            for e in range(M_GROUPS):
                for k in range(K_TILES):
                    xst = xs_pool.tile([P, ME], BF, tag="xs",
                                       name=f"xs_{e}_{k}")
                    nc.scalar.dma_start(
                        out=xst[:],
                        in_=xT_ap[bass.ts(k, P), bass.ts(e, ME)])
                    xs_tiles[e][k] = xst

            for n in range(N_CHUNKS):
                ws_tiles = []
                for k in range(K_TILES):
                    wst = ws_pool.tile([P, N_CHUNK], BF, tag="ws",
                                       name=f"ws_{n}_{k}")
                    # chunk 0 paces the PE k-loop: feed it from two queues
                    eng = nc.sync if (n == 0 and k % 2 == 1) else nc.gpsimd
                    eng.dma_start(
                        out=wst[:],
                        in_=wT_ap[bass.ts(n * K_TILES + k, P), :])
                    ws_tiles.append(wst)

                for g in range(M_GROUPS):
                    ps = [[ps_pool.tile([P, NH], F32, tag="ps",
                                        name=f"ps_{n}_{g}_{mi}_{h}")
                           for h in range(2)] for mi in range(MG)]
                    for k in range(K_TILES):
                        for mi in range(MG):
                            for h in range(2):
                                nc.tensor.matmul(
                                    ps[mi][h][:],
                                    lhsT=xs_tiles[g][k][:, bass.ts(mi, P)],
                                    rhs=ws_tiles[k][:, bass.ts(h, NH)],
                                    start=(k == 0),
                                    stop=(k == K_TILES - 1),
                                )
                    for mi in range(MG):
                        for h in range(2):
                            m_tile = g * MG + mi
                            nh = n * 2 + h
                            st = stage_pool.tile([P, NH], BF, tag="st",
                                                 name=f"st_{n}_{g}_{mi}_{h}")
                            nc.vector.tensor_copy(out=st[:],
                                                  in_=ps[mi][h][:])
                            nc.sync.dma_start(
                                out=out_ap[
                                    bass.ts(m_tile * (OUT_F // NH) + nh, P),
                                    :],
                                in_=st[:])

    nc.compile()
    return nc


def _get_nc():
    if "nc" not in _CACHE:
        _CACHE["nc"] = _build()
    return _CACHE["nc"]


def _marshal(x, weight, lora_P, lora_sigma, lora_Q):
    import ml_dtypes

    bf16 = ml_dtypes.bfloat16
    W = np.asarray(weight, dtype=np.float32)
    Ps = np.asarray(lora_P, dtype=np.float32) * np.asarray(
        lora_sigma, dtype=np.float32)[None, :]
    Wm = W + SCALING * (Ps @ np.asarray(lora_Q, dtype=np.float32).T)
    # [in, out] -> [n, k, p, c] -> 2D so each [128, 1024] strip is contiguous
    wT_np = np.ascontiguousarray(
        Wm.T.astype(bf16)
        .reshape(K_TILES, P, N_CHUNKS, N_CHUNK)
        .transpose(2, 0, 1, 3)
        .reshape(N_CHUNKS * K_TILES * P, N_CHUNK))
    X = np.asarray(x, dtype=np.float32).reshape(M_TOTAL, IN_F)
    in_maps = []
    for c in range(N_CORES):
        xT_np = np.ascontiguousarray(
            X[c * M_CORE:(c + 1) * M_CORE].T.astype(bf16))
        in_maps.append({"xT": xT_np, "wT": wT_np})
    return in_maps


def _gather(res):
    outs = []
    for c in range(N_CORES):
        o = np.asarray(res.results[c]["out"]).astype(np.float32)
        outs.append(
            o.reshape(M_TILES, OUT_F // NH, P, NH)
            .transpose(0, 2, 1, 3)
            .reshape(M_CORE, OUT_F))
    return np.concatenate(outs, axis=0).reshape(B, S, OUT_F)


def kernel(x, weight, lora_P, lora_sigma, lora_Q):
    from concourse.bass_utils import run_bass_kernel_spmd

    nc = _get_nc()
    in_maps = _marshal(x, weight, lora_P, lora_sigma, lora_Q)
    res = run_bass_kernel_spmd(nc, in_maps, core_ids=list(range(N_CORES)))
    return _gather(res)


# revision 8
# speedup vs baseline: 1.3507x; 1.1379x over previous
"""AdaLoRA dense MLP kernel for 8 TRN2 NeuronCores.

out = x @ (W + (alpha/r) * (P * sigma) @ Q^T)^T

The rank-12 LoRA delta is merged into W on the host (~400 MFLOP of
numpy), so the device runs one dense GEMM per core, data-parallel over
the 16384 token rows (2048/core, no collectives).

Mixed precision: the first 8 of 32 K-tiles run as fp8(e4m3) DoubleRow
matmuls (K=256/instruction at the same 216 ns as a bf16 K=128
instruction -> 2x), the rest in bf16. Measured full-tensor rel err
1.59e-2 vs the 2e-2 gate (bf16-only is 2.6e-3; fp8 e4m3 everywhere
would be 3.1e-2). W is pre-scaled by 2^12 on the host so its e4m3
encoding stays in normal range; evictions multiply by 2^-12.

Per-core blocking (M=2048, K=4096, N=4096), x stationary / W moving:
  x resident in SBUF: per eighth (256 rows of M): 4 fp8 pair-strips
  [128, 2, 256] + 24 bf16 strips [128, 256] (112 KB/partition)
  for n in 4:               # 1024-wide output chunks; W strips stream
    for g in groups:        # 2 m-tiles x 2 n-halves -> 4 PSUM banks,
                            # ping-pong so evictions overlap fully
                            # (chunk 0 leads with a 4-m-tile group so its
                            # k-step needs only ~222 GB/s of DMA feed)
      4 DR pair-steps + 24 bf16 k-steps; each stationary x-tile is
      reused for both n-halves (an un-amortized LDWEIGHTS adds +43ns
      to every bf16 matmul: measured 259ns vs the 216ns floor)
      evict: tensor_scalar_mul 2^-12 PSUM -> SBUF bf16, DMA on sync
Total 3584 matmuls x 216 ns = 774 us of PE time.

DMA queues: x strips on scalar only (the eviction path must not queue
behind them); W strips on gpsimd, split with sync for chunk 0 (which
paces the PE); output on sync; all evictions on vector except the last
chunk (vector+scalar) to shorten the tail.
"""

import numpy as np

B, S, IN_F, OUT_F, RANK = 4, 4096, 4096, 4096, 12
SCALING = 16.0 / 12.0
N_CORES = 8
M_TOTAL = B * S               # 16384
M_CORE = M_TOTAL // N_CORES   # 2048

P = 128                       # partitions
K_TILES = IN_F // P           # 32
KF_PAIRS = 4                  # fp8 DoubleRow K-pairs (k-tiles 0..7)
KB_TILES = K_TILES - 2 * KF_PAIRS  # 24 bf16 k-tiles (8..31)
N_CHUNK = 1024                # ws chunk width
N_CHUNKS = OUT_F // N_CHUNK   # 4
NH = 512                      # psum half width
M_TILES = M_CORE // P         # 16
M_EIGHTHS = 8
ME = 256                      # x strip width (2 m-tiles)
WSCALE = 2.0 ** 12

_CACHE = {}


def _build():
    import concourse.bass as bass
    import concourse.tile as tile
    from concourse import bacc, mybir

    BF = mybir.dt.bfloat16
    F8 = mybir.dt.float8e4
    F32 = mybir.dt.float32
    DR = mybir.MatmulPerfMode.DoubleRow

    nc = bacc.Bacc("TRN2", target_bir_lowering=False, debug=False,
                   num_devices=N_CORES)

    # bf16 x^T (only k-tiles 8..31 are read; full shape keeps marshal simple)
    xT = nc.declare_dram_parameter("xT", [IN_F, M_CORE], BF, isOutput=False)
    # fp8 x pair-strips: row ((e*4 + kk)*128 + p) -> [2, 256]
    x8 = nc.declare_dram_parameter(
        "x8", [M_EIGHTHS * KF_PAIRS * P, 2, ME], F8, isOutput=False)
    # bf16 W strips (k >= 8): row ((n*24 + (k-8))*128 + p) -> [1024]
    wT = nc.declare_dram_parameter(
        "wT", [N_CHUNKS * KB_TILES * P, N_CHUNK], BF, isOutput=False)
    # fp8 W pair-strips: row ((n*4 + kk)*128 + p) -> [2, 1024]
    w8 = nc.declare_dram_parameter(
        "w8", [N_CHUNKS * KF_PAIRS * P, 2, N_CHUNK], F8, isOutput=False)
    # out row ((mt*8 + nh)*128 + p), col c -> out[mt*128+p, nh*512+c], bf16
    out = nc.declare_dram_parameter(
        "out", [M_TILES * (OUT_F // NH) * P, NH], BF, isOutput=True)

    xT_ap, x8_ap, wT_ap, w8_ap, out_ap = (
        xT.ap(), x8.ap(), wT.ap(), w8.ap(), out.ap())

    with tile.TileContext(nc) as tc:
        with tc.tile_pool(name="xs", bufs=M_EIGHTHS * KB_TILES) as xs_pool, \
             tc.tile_pool(name="x8", bufs=M_EIGHTHS * KF_PAIRS) as x8_pool, \
             tc.tile_pool(name="ws", bufs=KB_TILES + 2) as ws_pool, \
             tc.tile_pool(name="w8", bufs=KF_PAIRS + 2) as w8_pool, \
             tc.tile_pool(name="stage", bufs=12) as stage_pool, \
             tc.tile_pool(name="ps", bufs=8, space="PSUM") as ps_pool:

            # resident x on the scalar queue, in need order: chunk 0's
            # first group uses eighths 0+1, so interleave those two, then
            # stream eighths 2..7
            x8_tiles = [[None] * KF_PAIRS for _ in range(M_EIGHTHS)]
            xs_tiles = [[None] * KB_TILES for _ in range(M_EIGHTHS)]

            def emit_x(e):
                for kk in range(KF_PAIRS):
                    t = x8_pool.tile([P, 2, ME], F8, tag="x8",
                                     name=f"x8_{e}_{kk}")
                    nc.scalar.dma_start(
                        out=t[:],
                        in_=x8_ap[bass.ts(e * KF_PAIRS + kk, P), :, :])
                    x8_tiles[e][kk] = t
                for k in range(KB_TILES):
                    t = xs_pool.tile([P, ME], BF, tag="xs",
                                     name=f"xs_{e}_{k}")
                    nc.scalar.dma_start(
                        out=t[:],
                        in_=xT_ap[bass.ts(2 * KF_PAIRS + k, P),
                                  bass.ts(e, ME)])
                    xs_tiles[e][k] = t

            for kk in range(KF_PAIRS):      # interleave e0/e1 pair strips
                for e in (0, 1):
                    t = x8_pool.tile([P, 2, ME], F8, tag="x8",
                                     name=f"x8_{e}_{kk}")
                    nc.scalar.dma_start(
                        out=t[:],
                        in_=x8_ap[bass.ts(e * KF_PAIRS + kk, P), :, :])
                    x8_tiles[e][kk] = t
            for k in range(KB_TILES):
                for e in (0, 1):
                    t = xs_pool.tile([P, ME], BF, tag="xs",
                                     name=f"xs_{e}_{k}")
                    nc.scalar.dma_start(
                        out=t[:],
                        in_=xT_ap[bass.ts(2 * KF_PAIRS + k, P),
                                  bass.ts(e, ME)])
                    xs_tiles[e][k] = t
            for e in range(2, M_EIGHTHS):
                emit_x(e)

            for n in range(N_CHUNKS):
                w8_tiles = []
                ws_tiles = []
                for kk in range(KF_PAIRS):
                    t = w8_pool.tile([P, 2, N_CHUNK], F8, tag="w8",
                                     name=f"w8_{n}_{kk}")
                    eng = nc.sync if (n == 0 and kk % 2 == 1) else nc.gpsimd
                    eng.dma_start(
                        out=t[:],
                        in_=w8_ap[bass.ts(n * KF_PAIRS + kk, P), :, :])
                    w8_tiles.append(t)
                for k in range(KB_TILES):
                    t = ws_pool.tile([P, N_CHUNK], BF, tag="ws",
                                     name=f"ws_{n}_{k}")
                    eng = nc.sync if (n == 0 and k % 2 == 1) else nc.gpsimd
                    eng.dma_start(
                        out=t[:],
                        in_=wT_ap[bass.ts(n * KB_TILES + k, P), :])
                    ws_tiles.append(t)

                # chunk 0 leads with a 4-m-tile group (8 banks): its k-step
                # is 2x longer, halving the DMA feed rate it needs while
                # the input stream ramps
                groups = ([(0, 4)] + [(m, 2) for m in range(4, M_TILES, 2)]
                          if n == 0
                          else [(m, 2) for m in range(0, M_TILES, 2)])
                for m0, mg in groups:
                    ps = [[ps_pool.tile([P, NH], F32, tag="ps",
                                        name=f"ps_{n}_{m0}_{mi}_{h}")
                           for h in range(2)] for mi in range(mg)]
                    for kk in range(KF_PAIRS):
                        for mi in range(mg):
                            xt = x8_tiles[(m0 + mi) // 2][kk]
                            sub = (m0 + mi) % 2
                            for h in range(2):
                                nc.tensor.matmul(
                                    ps[mi][h][:],
                                    lhsT=xt[:, :, bass.ts(sub, P)],
                                    rhs=w8_tiles[kk][:, :, bass.ts(h, NH)],
                                    start=(kk == 0),
                                    stop=False,
                                    perf_mode=DR,
                                )
                    for k in range(KB_TILES):
                        for mi in range(mg):
                            xt = xs_tiles[(m0 + mi) // 2][k]
                            sub = (m0 + mi) % 2
                            for h in range(2):
                                nc.tensor.matmul(
                                    ps[mi][h][:],
                                    lhsT=xt[:, bass.ts(sub, P)],
                                    rhs=ws_tiles[k][:, bass.ts(h, NH)],
                                    start=False,
                                    stop=(k == KB_TILES - 1),
                                )
                    for mi in range(mg):
                        for h in range(2):
                            m_tile = m0 + mi
                            nh = n * 2 + h
                            st = stage_pool.tile([P, NH], BF, tag="st",
                                                 name=f"st_{n}_{m_tile}_{h}")
                            # un-scale the 2^12 weight pre-scale on evict;
                            # last chunk splits engines to shorten the tail
                            if n == N_CHUNKS - 1 and (mi + h) % 2 == 1:
                                nc.scalar.mul(st[:], ps[mi][h][:],
                                              1.0 / WSCALE)
                            else:
                                nc.vector.tensor_scalar_mul(
                                    st[:], ps[mi][h][:], 1.0 / WSCALE)
                            nc.sync.dma_start(
                                out=out_ap[
                                    bass.ts(m_tile * (OUT_F // NH) + nh, P),
                                    :],
                                in_=st[:])

    nc.compile()
    return nc


def _get_nc():
    if "nc" not in _CACHE:
        _CACHE["nc"] = _build()
    return _CACHE["nc"]


def _marshal(x, weight, lora_P, lora_sigma, lora_Q):
    import ml_dtypes

    bf16 = ml_dtypes.bfloat16
    f8 = ml_dtypes.float8_e4m3
    W = np.asarray(weight, dtype=np.float32)
    Ps = np.asarray(lora_P, dtype=np.float32) * np.asarray(
        lora_sigma, dtype=np.float32)[None, :]
    Wm = W + SCALING * (Ps @ np.asarray(lora_Q, dtype=np.float32).T)
    WsT = np.ascontiguousarray(Wm.T) * np.float32(WSCALE)  # [in, out]
    KF = 2 * KF_PAIRS * P                                  # 1024
    # fp8 W pair-strips [n, kk, p, j, c]
    w8_np = np.ascontiguousarray(
        WsT[:KF].astype(f8)
        .reshape(KF_PAIRS, 2, P, N_CHUNKS, N_CHUNK)
        .transpose(3, 0, 2, 1, 4)
        .reshape(N_CHUNKS * KF_PAIRS * P, 2, N_CHUNK))
    # bf16 W strips for k >= 8: [n, k, p, c]
    wT_np = np.ascontiguousarray(
        WsT[KF:].astype(bf16)
        .reshape(KB_TILES, P, N_CHUNKS, N_CHUNK)
        .transpose(2, 0, 1, 3)
        .reshape(N_CHUNKS * KB_TILES * P, N_CHUNK))
    X = np.asarray(x, dtype=np.float32).reshape(M_TOTAL, IN_F)
    in_maps = []
    for c in range(N_CORES):
        Xc = np.ascontiguousarray(X[c * M_CORE:(c + 1) * M_CORE].T)
        xT_np = Xc.astype(bf16)
        x8_np = np.ascontiguousarray(
            Xc[:KF].astype(f8)
            .reshape(KF_PAIRS, 2, P, M_EIGHTHS, ME)
            .transpose(3, 0, 2, 1, 4)
            .reshape(M_EIGHTHS * KF_PAIRS * P, 2, ME))
        in_maps.append({"xT": xT_np, "x8": x8_np,
                        "wT": wT_np, "w8": w8_np})
    return in_maps


def _gather(res):
    outs = []
    for c in range(N_CORES):
        o = np.asarray(res.results[c]["out"]).astype(np.float32)
        outs.append(
            o.reshape(M_TILES, OUT_F // NH, P, NH)
            .transpose(0, 2, 1, 3)
            .reshape(M_CORE, OUT_F))
    return np.concatenate(outs, axis=0).reshape(B, S, OUT_F)


def kernel(x, weight, lora_P, lora_sigma, lora_Q):
    from concourse.bass_utils import run_bass_kernel_spmd

    nc = _get_nc()
    in_maps = _marshal(x, weight, lora_P, lora_sigma, lora_Q)
    res = run_bass_kernel_spmd(nc, in_maps, core_ids=list(range(N_CORES)))
    return _gather(res)


# revision 9
# speedup vs baseline: 1.4007x; 1.0370x over previous
"""AdaLoRA dense MLP kernel for 8 TRN2 NeuronCores.

out = x @ (W + (alpha/r) * (P * sigma) @ Q^T)^T

The rank-12 LoRA delta is merged into W on the host (~400 MFLOP of
numpy), so the device runs one dense GEMM per core, data-parallel over
the 16384 token rows (2048/core, no collectives).

Mixed precision: the first 8 of 32 K-tiles run as fp8(e4m3) DoubleRow
matmuls (K=256/instruction at the same 216 ns as a bf16 K=128
instruction -> 2x), the rest in bf16. Measured full-tensor rel err
1.59e-2 vs the 2e-2 gate (bf16-only is 2.6e-3; fp8 e4m3 everywhere
would be 3.1e-2). W is pre-scaled by 2^12 on the host so its e4m3
encoding stays in normal range; evictions multiply by 2^-12.

Per-core blocking (M=2048, K=4096, N=4096), x stationary / W moving:
  x resident in SBUF: per eighth (256 rows of M): 4 fp8 pair-strips
  [128, 2, 256] + 24 bf16 strips [128, 256] (112 KB/partition)
  for n in 4:               # 1024-wide output chunks; W strips stream
    for g in groups:        # 2 m-tiles x 2 n-halves -> 4 PSUM banks,
                            # ping-pong so evictions overlap fully
                            # (chunk 0 leads with a 4-m-tile group so its
                            # k-step needs only ~222 GB/s of DMA feed)
      4 DR pair-steps + 24 bf16 k-steps; each stationary x-tile is
      reused for both n-halves (an un-amortized LDWEIGHTS adds +43ns
      to every bf16 matmul: measured 259ns vs the 216ns floor)
      evict: tensor_scalar_mul 2^-12 PSUM -> SBUF bf16, DMA on sync
Total 3584 matmuls x 216 ns = 774 us of PE time.

DMA queues: x strips on scalar only (the eviction path must not queue
behind them); W strips on gpsimd, split with sync for chunk 0 (which
paces the PE); output on sync; all evictions on vector except the last
chunk (vector+scalar) to shorten the tail.
"""

import numpy as np

B, S, IN_F, OUT_F, RANK = 4, 4096, 4096, 4096, 12
SCALING = 16.0 / 12.0
N_CORES = 8
M_TOTAL = B * S               # 16384
M_CORE = M_TOTAL // N_CORES   # 2048

P = 128                       # partitions
K_TILES = IN_F // P           # 32
KF_PAIRS = 5                  # fp8 DoubleRow K-pairs (k-tiles 0..9)
KB_TILES = K_TILES - 2 * KF_PAIRS  # 22 bf16 k-tiles (10..31)
N_CHUNK = 1024                # ws chunk width
N_CHUNKS = OUT_F // N_CHUNK   # 4
NH = 512                      # psum half width
M_TILES = M_CORE // P         # 16
M_EIGHTHS = 8
ME = 256                      # x strip width (2 m-tiles)
WSCALE = 2.0 ** 12

_CACHE = {}


def _build():
    import concourse.bass as bass
    import concourse.tile as tile
    from concourse import bacc, mybir

    BF = mybir.dt.bfloat16
    F8 = mybir.dt.float8e4
    F32 = mybir.dt.float32
    DR = mybir.MatmulPerfMode.DoubleRow

    nc = bacc.Bacc("TRN2", target_bir_lowering=False, debug=False,
                   num_devices=N_CORES)

    # bf16 x^T (only k-tiles 8..31 are read; full shape keeps marshal simple)
    xT = nc.declare_dram_parameter("xT", [IN_F, M_CORE], BF, isOutput=False)
    # fp8 x pair-strips: row ((e*4 + kk)*128 + p) -> [2, 256]
    x8 = nc.declare_dram_parameter(
        "x8", [M_EIGHTHS * KF_PAIRS * P, 2, ME], F8, isOutput=False)
    # bf16 W strips (k >= 8): row ((n*24 + (k-8))*128 + p) -> [1024]
    wT = nc.declare_dram_parameter(
        "wT", [N_CHUNKS * KB_TILES * P, N_CHUNK], BF, isOutput=False)
    # fp8 W pair-strips: row ((n*4 + kk)*128 + p) -> [2, 1024]
    w8 = nc.declare_dram_parameter(
        "w8", [N_CHUNKS * KF_PAIRS * P, 2, N_CHUNK], F8, isOutput=False)
    # out row ((mt*8 + nh)*128 + p), col c -> out[mt*128+p, nh*512+c], bf16
    out = nc.declare_dram_parameter(
        "out", [M_TILES * (OUT_F // NH) * P, NH], BF, isOutput=True)

    xT_ap, x8_ap, wT_ap, w8_ap, out_ap = (
        xT.ap(), x8.ap(), wT.ap(), w8.ap(), out.ap())

    with tile.TileContext(nc) as tc:
        with tc.tile_pool(name="xs", bufs=M_EIGHTHS * KB_TILES) as xs_pool, \
             tc.tile_pool(name="x8", bufs=M_EIGHTHS * KF_PAIRS) as x8_pool, \
             tc.tile_pool(name="ws", bufs=KB_TILES + 2) as ws_pool, \
             tc.tile_pool(name="w8", bufs=KF_PAIRS + 2) as w8_pool, \
             tc.tile_pool(name="stage", bufs=12) as stage_pool, \
             tc.tile_pool(name="ps", bufs=8, space="PSUM") as ps_pool:

            # resident x on the scalar queue, in need order: chunk 0's
            # first group uses eighths 0+1, so interleave those two, then
            # stream eighths 2..7
            x8_tiles = [[None] * KF_PAIRS for _ in range(M_EIGHTHS)]
            xs_tiles = [[None] * KB_TILES for _ in range(M_EIGHTHS)]

            def emit_x(e):
                for kk in range(KF_PAIRS):
                    t = x8_pool.tile([P, 2, ME], F8, tag="x8",
                                     name=f"x8_{e}_{kk}")
                    nc.scalar.dma_start(
                        out=t[:],
                        in_=x8_ap[bass.ts(e * KF_PAIRS + kk, P), :, :])
                    x8_tiles[e][kk] = t
                for k in range(KB_TILES):
                    t = xs_pool.tile([P, ME], BF, tag="xs",
                                     name=f"xs_{e}_{k}")
                    nc.scalar.dma_start(
                        out=t[:],
                        in_=xT_ap[bass.ts(2 * KF_PAIRS + k, P),
                                  bass.ts(e, ME)])
                    xs_tiles[e][k] = t

            for kk in range(KF_PAIRS):      # interleave e0/e1 pair strips
                for e in (0, 1):
                    t = x8_pool.tile([P, 2, ME], F8, tag="x8",
                                     name=f"x8_{e}_{kk}")
                    nc.scalar.dma_start(
                        out=t[:],
                        in_=x8_ap[bass.ts(e * KF_PAIRS + kk, P), :, :])
                    x8_tiles[e][kk] = t
            for k in range(KB_TILES):
                for e in (0, 1):
                    t = xs_pool.tile([P, ME], BF, tag="xs",
                                     name=f"xs_{e}_{k}")
                    nc.scalar.dma_start(
                        out=t[:],
                        in_=xT_ap[bass.ts(2 * KF_PAIRS + k, P),
                                  bass.ts(e, ME)])
                    xs_tiles[e][k] = t
            for e in range(2, M_EIGHTHS):
                emit_x(e)

            for n in range(N_CHUNKS):
                w8_tiles = []
                ws_tiles = []
                for kk in range(KF_PAIRS):
                    t = w8_pool.tile([P, 2, N_CHUNK], F8, tag="w8",
                                     name=f"w8_{n}_{kk}")
                    eng = nc.sync if (n == 0 and kk % 2 == 1) else nc.gpsimd
                    eng.dma_start(
                        out=t[:],
                        in_=w8_ap[bass.ts(n * KF_PAIRS + kk, P), :, :])
                    w8_tiles.append(t)
                for k in range(KB_TILES):
                    t = ws_pool.tile([P, N_CHUNK], BF, tag="ws",
                                     name=f"ws_{n}_{k}")
                    eng = nc.sync if (n == 0 and k % 2 == 1) else nc.gpsimd
                    eng.dma_start(
                        out=t[:],
                        in_=wT_ap[bass.ts(n * KB_TILES + k, P), :])
                    ws_tiles.append(t)

                # chunk 0 leads with a 4-m-tile group (8 banks): its k-step
                # is 2x longer, halving the DMA feed rate it needs while
                # the input stream ramps
                groups = ([(0, 4)] + [(m, 2) for m in range(4, M_TILES, 2)]
                          if n == 0
                          else [(m, 2) for m in range(0, M_TILES, 2)])
                for m0, mg in groups:
                    ps = [[ps_pool.tile([P, NH], F32, tag="ps",
                                        name=f"ps_{n}_{m0}_{mi}_{h}")
                           for h in range(2)] for mi in range(mg)]
                    for kk in range(KF_PAIRS):
                        for mi in range(mg):
                            xt = x8_tiles[(m0 + mi) // 2][kk]
                            sub = (m0 + mi) % 2
                            for h in range(2):
                                nc.tensor.matmul(
                                    ps[mi][h][:],
                                    lhsT=xt[:, :, bass.ts(sub, P)],
                                    rhs=w8_tiles[kk][:, :, bass.ts(h, NH)],
                                    start=(kk == 0),
                                    stop=False,
                                    perf_mode=DR,
                                )
                    for k in range(KB_TILES):
                        for mi in range(mg):
                            xt = xs_tiles[(m0 + mi) // 2][k]
                            sub = (m0 + mi) % 2
                            for h in range(2):
                                nc.tensor.matmul(
                                    ps[mi][h][:],
                                    lhsT=xt[:, bass.ts(sub, P)],
                                    rhs=ws_tiles[k][:, bass.ts(h, NH)],
                                    start=False,
                                    stop=(k == KB_TILES - 1),
                                )
                    for mi in range(mg):
                        for h in range(2):
                            m_tile = m0 + mi
                            nh = n * 2 + h
                            st = stage_pool.tile([P, NH], BF, tag="st",
                                                 name=f"st_{n}_{m_tile}_{h}")
                            # un-scale the 2^12 weight pre-scale on evict;
                            # last chunk splits engines to shorten the tail
                            if n == N_CHUNKS - 1 and (mi + h) % 2 == 1:
                                nc.scalar.mul(st[:], ps[mi][h][:],
                                              1.0 / WSCALE)
                            else:
                                nc.vector.tensor_scalar_mul(
                                    st[:], ps[mi][h][:], 1.0 / WSCALE)
                            nc.sync.dma_start(
                                out=out_ap[
                                    bass.ts(m_tile * (OUT_F // NH) + nh, P),
                                    :],
                                in_=st[:])

    nc.compile()
    return nc


def _get_nc():
    if "nc" not in _CACHE:
        _CACHE["nc"] = _build()
    return _CACHE["nc"]


def _marshal(x, weight, lora_P, lora_sigma, lora_Q):
    import ml_dtypes

    bf16 = ml_dtypes.bfloat16
    f8 = ml_dtypes.float8_e4m3
    W = np.asarray(weight, dtype=np.float32)
    Ps = np.asarray(lora_P, dtype=np.float32) * np.asarray(
        lora_sigma, dtype=np.float32)[None, :]
    Wm = W + SCALING * (Ps @ np.asarray(lora_Q, dtype=np.float32).T)
    WsT = np.ascontiguousarray(Wm.T) * np.float32(WSCALE)  # [in, out]
    KF = 2 * KF_PAIRS * P                                  # 1024
    # fp8 W pair-strips [n, kk, p, j, c]
    w8_np = np.ascontiguousarray(
        WsT[:KF].astype(f8)
        .reshape(KF_PAIRS, 2, P, N_CHUNKS, N_CHUNK)
        .transpose(3, 0, 2, 1, 4)
        .reshape(N_CHUNKS * KF_PAIRS * P, 2, N_CHUNK))
    # bf16 W strips for k >= 8: [n, k, p, c]
    wT_np = np.ascontiguousarray(
        WsT[KF:].astype(bf16)
        .reshape(KB_TILES, P, N_CHUNKS, N_CHUNK)
        .transpose(2, 0, 1, 3)
        .reshape(N_CHUNKS * KB_TILES * P, N_CHUNK))
    X = np.asarray(x, dtype=np.float32).reshape(M_TOTAL, IN_F)
    in_maps = []
    for c in range(N_CORES):
        Xc = np.ascontiguousarray(X[c * M_CORE:(c + 1) * M_CORE].T)
        xT_np = Xc.astype(bf16)
        x8_np = np.ascontiguousarray(
            Xc[:KF].astype(f8)
            .reshape(KF_PAIRS, 2, P, M_EIGHTHS, ME)
            .transpose(3, 0, 2, 1, 4)
            .reshape(M_EIGHTHS * KF_PAIRS * P, 2, ME))
        in_maps.append({"xT": xT_np, "x8": x8_np,
                        "wT": wT_np, "w8": w8_np})
    return in_maps


def _gather(res):
    outs = []
    for c in range(N_CORES):
        o = np.asarray(res.results[c]["out"]).astype(np.float32)
        outs.append(
            o.reshape(M_TILES, OUT_F // NH, P, NH)
            .transpose(0, 2, 1, 3)
            .reshape(M_CORE, OUT_F))
    return np.concatenate(outs, axis=0).reshape(B, S, OUT_F)


def kernel(x, weight, lora_P, lora_sigma, lora_Q):
    from concourse.bass_utils import run_bass_kernel_spmd

    nc = _get_nc()
    in_maps = _marshal(x, weight, lora_P, lora_sigma, lora_Q)
    res = run_bass_kernel_spmd(nc, in_maps, core_ids=list(range(N_CORES)))
    return _gather(res)
